# revision 16
# baseline (speedup 1.0000x reference)
"""Trainium2 Bass kernel for a contextual loss (cosine-distance softmin loss).

Math (per batch b):
  mu_c      = mean_n Y[b,c,n]
  xc = X-mu, yc = Y-mu                      (centered, [C,N])
  t[i,j]    = <xc_i, yc_j/||yc_j||>         (bf16 matmul, K=C=64)
  s[i,j]    = rx_i * t[i,j]                 (rx = 1/||xc_i||)
  pm_i      = max_j t[i,j]
  a_i       = rx5_i / (1.001 - 5*rx5_i*pm_i)     (rx5 = 0.2*rx)
  S'_i      = sum_j exp(a_i*(t_ij - pm_i))
  loss_b    = -log(mean_i 1/S'_i)

Sharding: 8 cores = 4 batches x 2 row-halves. Each core gets its full-batch
Y [64,4096] and its half of X's columns [64,2048], returns S' as [128,16]
(partition p, chunk k  <->  row k*128+p). Host reduces to the [4] loss.

Layout trick: X and Y are DMA'd TWICE, into partitions 0-63 and 64-127.
All elementwise setup ops then run on 128 partitions at the same cost as
64, and the duplicated halves feed the PE row-tiling: two 128-row chunks
run CONCURRENTLY as K=64 sub-matmuls at tile_position (0,0) / (64,0),
sharing one weight-load slot each, which keeps the PE dense.

Column norms without pre-centering: ||y_j - mu||^2 = colsum(y^2) - 2 mu^T y_j
(+ ||mu||^2, ~2e-4 relative, dropped), via PSUM accumulation of two bf16
matmuls: (0.5*ones)^T @ y^2  then  (-mu)^T @ y  over the duplicated rows.

On-device pipeline per 128-row chunk:
  PE   : 4 bf16 matmul slots (2 chunks in parallel, K=64, N=512)
  DVE  : TENSOR_MASK_REDUCE fuses PSUM->SBUF copy with a running row-max
  GPSIMD + DVE: tiny per-row chain  den -> 1/den -> aa -> bb
  ACT  : one exp(aa*t + bb) over [128,4096] with accumulated row-sum
"""

import math

import numpy as np

import concourse.bacc as bacc
import concourse.mybir as mybir
from concourse.dve_ops import TENSOR_MASK_REDUCE
from concourse.bass_utils import run_bass_kernel_spmd
from concourse.mybir import ActivationFunctionType as AF, AluOpType as OP, AxisListType
from concourse.tile import TileContext

F32 = mybir.dt.float32
BF16 = mybir.dt.bfloat16

B, C, N = 4, 64, 4096          # batch, channels, spatial (64*64)
NX = N // 2                    # rows per core (half batch)
CH = NX // 128                 # 16 chunks of 128 rows
HALF = N // 2                  # column half processed per DVE op
H_BAND = 5.0
EPS_MIN = 1e-3
LN02 = math.log(0.2)           # fold the 1/H into rx via exp(... + ln(1/H))

_NC_CACHE = {}


def build_nc():
    nc = bacc.Bacc("TRN2", target_bir_lowering=False, debug=False, num_devices=8)
    x_d = nc.dram_tensor("Xh", [C, NX], F32, kind="ExternalInput")
    y_d = nc.dram_tensor("Yb", [C, N], F32, kind="ExternalInput")
    out_d = nc.dram_tensor("out", [128, 2 * CH], F32, kind="ExternalOutput")

    with TileContext(nc) as tc:
        with (
            tc.tile_pool(name="persist", bufs=1) as persist,
            tc.tile_pool(name="mm", bufs=1, space="PSUM") as mmpool,
            tc.tile_pool(name="rb", bufs=2) as rbpool,
            tc.tile_pool(name="es", bufs=2) as espool,
            tc.tile_pool(name="small", bufs=4) as small,
        ):
            # -------- load inputs (single copy, partitions 0..63) ------------
            x_sb = persist.tile([C, NX], F32)
            nc.sync.dma_start(out=x_sb[:], in_=x_d[:])
            y_sb = persist.tile([C, N], F32)
            NSL = 4
            SL = N // NSL
            for sl in range(NSL):
                c0 = sl * SL
                nc.sync.dma_start(out=y_sb[:, c0:c0 + SL], in_=y_d[:, c0:c0 + SL])

            ones_y = persist.tile([C, 128], BF16)
            nc.vector.memset(ones_y[:], 1.0)
            ones2 = persist.tile([C, 2], BF16)
            nc.vector.memset(ones2[:], 1.0)
            c3big = persist.tile([128, 1], F32)
            nc.gpsimd.memset(c3big[:], 1.0e9)

            # ---------------- y mean (overlapped with DMA slices) ------------
            mus = small.tile([C, NSL], F32, tag="mus")
            for sl in range(NSL):
                c0 = sl * SL
                nc.vector.reduce_sum(out=mus[:, sl:sl + 1],
                                     in_=y_sb[:, c0:c0 + SL], axis=AxisListType.X)
            musum = small.tile([C, 1], F32, tag="musum")
            nc.vector.reduce_sum(out=musum[:], in_=mus[:], axis=AxisListType.X)
            mu = small.tile([C, 1], F32, tag="mu")
            nc.vector.tensor_scalar_mul(mu[:], musum[:], 1.0 / N)

            # -2*mu broadcasts (bf16) for the norm-correction matmuls
            nmubc = persist.tile([C, 128], BF16)
            nc.vector.tensor_scalar(nmubc[:], ones_y[:], mu[:], -2.0,
                                    OP.mult, OP.mult)
            nmucol = persist.tile([C, 2], BF16)
            nc.vector.tensor_scalar(nmucol[:], ones2[:], mu[:], -2.0,
                                    OP.mult, OP.mult)
            ybf = persist.tile([C, N], BF16)
            nc.vector.tensor_copy(ybf[:], y_sb[:])

            # squares first (one ACT table set), y halves then x
            ysq = persist.tile([C, N], BF16)
            for sl in range(NSL):
                c0 = sl * SL
                nc.scalar.activation(
                    ysq[:, c0:c0 + SL], y_sb[:, c0:c0 + SL], AF.Square,
                )
            xsq = persist.tile([C, NX], BF16)
            nc.gpsimd.tensor_tensor(xsq[:], x_sb[:], x_sb[:], OP.mult)
            xcen = persist.tile([128, NX], BF16)
            nc.vector.tensor_scalar(xcen[:C, :], x_sb[:], mu[:], None, OP.subtract)
            nc.sync.dma_start(out=xcen[C:, :], in_=xcen[:C, :])

            # colsum matmuls: ny2 halves (PSUM-accumulated mu correction),
            # then nx2 in [128 rows, chunk] layout
            pss = []
            for h in range(2):
                ps = mmpool.tile([128, HALF], F32, tag="mmA" if h == 0 else "mmB",
                                 name="ps")
                for j in range(4):
                    c0 = h * HALF + j * 512
                    nc.tensor.matmul(
                        ps[:, j * 512:(j + 1) * 512],
                        lhsT=ones_y[:],
                        rhs=ysq[:, c0:c0 + 512],
                        start=True, stop=False,
                    )
                    nc.tensor.matmul(
                        ps[:, j * 512:(j + 1) * 512],
                        lhsT=nmubc[:],
                        rhs=ybf[:, c0:c0 + 512],
                        start=False, stop=True,
                    )
                pss.append(ps)
            nx2 = mmpool.tile([128, 2 * CH], F32, tag="mmB", name="nx2")
            for k in range(CH):
                nc.tensor.matmul(
                    nx2[:, 2 * k:2 * k + 2],
                    lhsT=xsq[:, k * 128:(k + 1) * 128],
                    rhs=ones2[:],
                    start=True, stop=False,
                )
                nc.tensor.matmul(
                    nx2[:, 2 * k:2 * k + 2],
                    lhsT=xcen[:C, k * 128:(k + 1) * 128],
                    rhs=nmucol[:],
                    start=False, stop=True,
                )

            # sqrt batch (second ACT table set), then DVE recip -> ry, yhat
            yhat = persist.tile([128, N], BF16)
            sqny = persist.tile([C, N], F32)
            ry_bc = persist.tile([C, N], F32)
            for h in range(2):
                nc.scalar.activation(sqny[:, h * HALF:(h + 1) * HALF],
                                     pss[h][:C, :], AF.Sqrt)
            sqnx = small.tile([128, CH], F32, tag="sqnx")
            nc.scalar.activation(
                sqnx[:], nx2[:].rearrange("p (k two) -> p k two", two=2)[:, :, 0],
                AF.Sqrt,
            )
            # cpm = -0.2002/rx5 = -1.001*sqrt(nx2); aa = -0.2/(pm + cpm)
            cpm = persist.tile([128, CH], F32)
            nc.vector.tensor_scalar_mul(cpm[:], sqnx[:], -(1.0 + EPS_MIN))
            for h in range(2):
                nc.vector.reciprocal_approx_fast(
                    ry_bc[:, h * HALF:(h + 1) * HALF],
                    sqny[:, h * HALF:(h + 1) * HALF])
                QW = HALF // 2
                for q in range(2):
                    c0 = h * HALF + q * QW
                    nc.vector.scalar_tensor_tensor(
                        yhat[:C, c0:c0 + QW],
                        in0=y_sb[:, c0:c0 + QW],
                        scalar=mu[:],
                        in1=ry_bc[:, c0:c0 + QW],
                        op0=OP.subtract,
                        op1=OP.mult,
                    )
                nc.sync.dma_start(out=yhat[C:, h * HALF:(h + 1) * HALF],
                                  in_=yhat[:C, h * HALF:(h + 1) * HALF])

            # ---------------- main loop: chunk pairs via PE row tiling --------
            ssums = persist.tile([128, 2 * CH], F32)
            for kp in range(CH // 2):
                kA, kB = 2 * kp, 2 * kp + 1
                lhsA = xcen[0:C, kA * 128:(kA + 1) * 128]
                lhsB = xcen[C:128, kB * 128:(kB + 1) * 128]
                pmP = small.tile([128, 4], F32, tag="pmP", name="pmP")
                rbA = rbpool.tile([128, N], F32, tag="rbA", name="rbA")
                rbB = rbpool.tile([128, N], F32, tag="rbB", name="rbB")
                rbts = [rbA, rbB]
                psAs, psBs = [], []
                for h in range(2):
                    psA = mmpool.tile([128, HALF], F32, tag="mmA", name="psA")
                    psB = mmpool.tile([128, HALF], F32, tag="mmB", name="psB")
                    for j in range(4):
                        c0 = h * HALF + j * 512
                        nc.tensor.matmul(
                            psA[:, j * 512:(j + 1) * 512],
                            lhsT=lhsA,
                            rhs=yhat[0:C, c0:c0 + 512],
                            start=True, stop=True,
                            tile_position=(0, 0),
                        )
                        nc.tensor.matmul(
                            psB[:, j * 512:(j + 1) * 512],
                            lhsT=lhsB,
                            rhs=yhat[C:128, c0:c0 + 512],
                            start=True, stop=True,
                            tile_position=(64, 0),
                        )
                    psAs.append(psA)
                    psBs.append(psB)

                for i, (k, pss, rbt) in enumerate(
                        ((kA, psAs, rbA), (kB, psBs, rbB))):
                    pm = pmP[:, 2 * i:2 * i + 2]
                    for h in range(2):
                        init = -3.0e38 if h == 0 else pm[:, 0:1]
                        # rb = copy(ps); pm = max(row-max(ps), init)
                        nc.vector._custom_dve(
                            TENSOR_MASK_REDUCE,
                            out=rbt[:, h * HALF:(h + 1) * HALF],
                            in0=pss[h][:],
                            in1=c3big[:],
                            s0=0.0,
                            s1=init,
                            imm2=1.0,
                            accum_out=pm[:, h:h + 1],
                        )
                    # per-row constants: aa = -0.2/(pm + c), c = -0.2002/rx5,
                    # bb = -aa*pm  (gpsimd + fast reciprocal on DVE)
                    pmx = pm[:, 1:2]
                    tden = small.tile([128, 1], F32, tag=f"tden{i}", name="tden")
                    nc.gpsimd.tensor_tensor(tden[:], pmx, cpm[:, k:k + 1], OP.add)
                    rec = small.tile([128, 1], F32, tag=f"rec{i}", name="rec")
                    nc.vector.reciprocal_approx_fast(rec[:], tden[:])
                    aa = small.tile([128, 1], F32, tag=f"aa{i}", name="aa")
                    nc.gpsimd.tensor_scalar_mul(aa[:], rec[:], -0.2)
                    bbt = small.tile([128, 1], F32, tag=f"bbt{i}", name="bbt")
                    nc.gpsimd.tensor_tensor(bbt[:], aa[:], pmx, OP.mult)
                    bb = small.tile([128, 1], F32, tag=f"bb{i}", name="bb")
                    nc.gpsimd.tensor_scalar_mul(bb[:], bbt[:], -1.0)
                    for h in range(2):
                        es = espool.tile([128, HALF], BF16, tag="es", name="es")
                        nc.scalar.activation(
                            es[:], rbt[:, h * HALF:(h + 1) * HALF], AF.Exp,
                            bias=bb[:], scale=aa[:],
                            accum_out=ssums[:, 2 * k + h:2 * k + h + 1],
                        )

            # ---------------- finalize ----------------
            nc.sync.dma_start(out=out_d[:], in_=ssums[:])

    nc.compile()
    return nc


def _get_nc():
    if "nc" not in _NC_CACHE:
        _NC_CACHE["nc"] = build_nc()
    return _NC_CACHE["nc"]


def make_in_maps(X_features, Y_features):
    X = np.ascontiguousarray(np.asarray(X_features, np.float32).reshape(B, C, N))
    Y = np.ascontiguousarray(np.asarray(Y_features, np.float32).reshape(B, C, N))
    in_maps = []
    for c in range(8):
        b, h = divmod(c, 2)
        in_maps.append({
            "Xh": np.ascontiguousarray(X[b, :, h * NX:(h + 1) * NX]),
            "Yb": Y[b],
        })
    return in_maps


def combine(results):
    """results: list of 8 dicts with 'out' [128, CH] = S' per row."""
    out = np.empty(B, np.float32)
    for b in range(B):
        tot = 0.0
        for h in range(2):
            s2 = results[2 * b + h]["out"].astype(np.float64)
            s = s2[:, 0::2] + s2[:, 1::2]
            tot += (1.0 / s).sum()
        out[b] = -np.log(tot / N)
    return out


def kernel(X_features, Y_features):
    nc = _get_nc()
    in_maps = make_in_maps(X_features, Y_features)
    res = run_bass_kernel_spmd(nc, in_maps, core_ids=list(range(8)))
    return combine(res.results)


if __name__ == "__main__":
    rng = np.random.default_rng(0)
    X = rng.standard_normal((B, C, 64, 64)).astype(np.float32)
    Y = rng.standard_normal((B, C, 64, 64)).astype(np.float32)
    print(kernel(X_features=X, Y_features=Y))


# revision 17
# speedup vs baseline: 1.0051x; 1.0051x over previous
"""Trainium2 Bass kernel for a contextual loss (cosine-distance softmin loss).

Math (per batch b):
  mu_c      = mean_n Y[b,c,n]
  xc = X-mu, yc = Y-mu                      (centered, [C,N])
  t[i,j]    = <xc_i, yc_j/||yc_j||>         (bf16 matmul, K=C=64)
  s[i,j]    = rx_i * t[i,j]                 (rx = 1/||xc_i||)
  pm_i      = max_j t[i,j]
  a_i       = rx5_i / (1.001 - 5*rx5_i*pm_i)     (rx5 = 0.2*rx)
  S'_i      = sum_j exp(a_i*(t_ij - pm_i))
  loss_b    = -log(mean_i 1/S'_i)

Sharding: 8 cores = 4 batches x 2 row-halves. Each core gets its full-batch
Y [64,4096] and its half of X's columns [64,2048], returns S' as [128,16]
(partition p, chunk k  <->  row k*128+p). Host reduces to the [4] loss.

Layout trick: X and Y are DMA'd TWICE, into partitions 0-63 and 64-127.
All elementwise setup ops then run on 128 partitions at the same cost as
64, and the duplicated halves feed the PE row-tiling: two 128-row chunks
run CONCURRENTLY as K=64 sub-matmuls at tile_position (0,0) / (64,0),
sharing one weight-load slot each, which keeps the PE dense.

Column norms without pre-centering: ||y_j - mu||^2 = colsum(y^2) - 2 mu^T y_j
(+ ||mu||^2, ~2e-4 relative, dropped), via PSUM accumulation of two bf16
matmuls: (0.5*ones)^T @ y^2  then  (-mu)^T @ y  over the duplicated rows.

On-device pipeline per 128-row chunk:
  PE   : 4 bf16 matmul slots (2 chunks in parallel, K=64, N=512)
  DVE  : TENSOR_MASK_REDUCE fuses PSUM->SBUF copy with a running row-max
  GPSIMD + DVE: tiny per-row chain  den -> 1/den -> aa -> bb
  ACT  : one exp(aa*t + bb) over [128,4096] with accumulated row-sum
"""

import math

import numpy as np

import concourse.bacc as bacc
import concourse.mybir as mybir
from concourse.dve_ops import TENSOR_MASK_REDUCE
from concourse.bass_utils import run_bass_kernel_spmd
from concourse.mybir import ActivationFunctionType as AF, AluOpType as OP, AxisListType
from concourse.tile import TileContext

F32 = mybir.dt.float32
BF16 = mybir.dt.bfloat16

B, C, N = 4, 64, 4096          # batch, channels, spatial (64*64)
NX = N // 2                    # rows per core (half batch)
CH = NX // 128                 # 16 chunks of 128 rows
HALF = N // 2                  # column half processed per DVE op
H_BAND = 5.0
EPS_MIN = 1e-3
LN02 = math.log(0.2)           # fold the 1/H into rx via exp(... + ln(1/H))

_NC_CACHE = {}


def build_nc():
    nc = bacc.Bacc("TRN2", target_bir_lowering=False, debug=False, num_devices=8)
    x_d = nc.dram_tensor("Xh", [C, NX], F32, kind="ExternalInput")
    y_d = nc.dram_tensor("Yb", [C, N], F32, kind="ExternalInput")
    out_d = nc.dram_tensor("out", [128, 2 * CH], F32, kind="ExternalOutput")

    with TileContext(nc) as tc:
        with (
            tc.tile_pool(name="persist", bufs=1) as persist,
            tc.tile_pool(name="mm", bufs=1, space="PSUM") as mmpool,
            tc.tile_pool(name="rb", bufs=2) as rbpool,
            tc.tile_pool(name="es", bufs=2) as espool,
            tc.tile_pool(name="small", bufs=4) as small,
        ):
            # -------- load inputs (single copy, partitions 0..63) ------------
            x_sb = persist.tile([C, NX], F32)
            nc.sync.dma_start(out=x_sb[:], in_=x_d[:])
            y_sb = persist.tile([C, N], F32)
            NSL = 4
            SL = N // NSL
            for sl in range(NSL):
                c0 = sl * SL
                nc.sync.dma_start(out=y_sb[:, c0:c0 + SL], in_=y_d[:, c0:c0 + SL])

            ones_y = persist.tile([C, 128], BF16)
            nc.vector.memset(ones_y[:], 1.0)
            ones2 = persist.tile([C, 2], BF16)
            nc.vector.memset(ones2[:], 1.0)
            c3big = persist.tile([128, 1], F32)
            nc.gpsimd.memset(c3big[:], 1.0e9)

            # ---------------- y mean (overlapped with DMA slices) ------------
            mus = small.tile([C, NSL], F32, tag="mus")
            for sl in range(NSL):
                c0 = sl * SL
                nc.vector.reduce_sum(out=mus[:, sl:sl + 1],
                                     in_=y_sb[:, c0:c0 + SL], axis=AxisListType.X)
            musum = small.tile([C, 1], F32, tag="musum")
            nc.vector.reduce_sum(out=musum[:], in_=mus[:], axis=AxisListType.X)
            mu = small.tile([C, 1], F32, tag="mu")
            nc.vector.tensor_scalar_mul(mu[:], musum[:], 1.0 / N)

            # -2*mu broadcasts (bf16) for the norm-correction matmuls
            nmubc = persist.tile([C, 128], BF16)
            nc.vector.tensor_scalar(nmubc[:], ones_y[:], mu[:], -2.0,
                                    OP.mult, OP.mult)
            nmucol = persist.tile([C, 2], BF16)
            nc.vector.tensor_scalar(nmucol[:], ones2[:], mu[:], -2.0,
                                    OP.mult, OP.mult)
            ybf = persist.tile([C, N], BF16)
            nc.vector.tensor_copy(ybf[:], y_sb[:])

            # squares first (one ACT table set), y halves then x
            ysq = persist.tile([C, N], BF16)
            for sl in range(NSL):
                c0 = sl * SL
                nc.scalar.activation(
                    ysq[:, c0:c0 + SL], y_sb[:, c0:c0 + SL], AF.Square,
                )
            xsq = persist.tile([C, NX], BF16)
            nc.gpsimd.tensor_tensor(xsq[:], x_sb[:], x_sb[:], OP.mult)
            xcen = persist.tile([128, NX], BF16)
            nc.vector.tensor_scalar(xcen[:C, :], x_sb[:], mu[:], None, OP.subtract)
            nc.sync.dma_start(out=xcen[C:, :], in_=xcen[:C, :])

            # colsum matmuls: ny2 halves (PSUM-accumulated mu correction),
            # then nx2 in [128 rows, chunk] layout
            pss = []
            for h in range(2):
                ps = mmpool.tile([128, HALF], F32, tag="mmA" if h == 0 else "mmB",
                                 name="ps")
                for j in range(4):
                    c0 = h * HALF + j * 512
                    nc.tensor.matmul(
                        ps[:, j * 512:(j + 1) * 512],
                        lhsT=ones_y[:],
                        rhs=ysq[:, c0:c0 + 512],
                        start=True, stop=False,
                    )
                    nc.tensor.matmul(
                        ps[:, j * 512:(j + 1) * 512],
                        lhsT=nmubc[:],
                        rhs=ybf[:, c0:c0 + 512],
                        start=False, stop=True,
                    )
                pss.append(ps)
            nx2 = mmpool.tile([128, 2 * CH], F32, tag="mmB", name="nx2")
            for k in range(CH):
                nc.tensor.matmul(
                    nx2[:, 2 * k:2 * k + 2],
                    lhsT=xsq[:, k * 128:(k + 1) * 128],
                    rhs=ones2[:],
                    start=True, stop=False,
                )
                nc.tensor.matmul(
                    nx2[:, 2 * k:2 * k + 2],
                    lhsT=xcen[:C, k * 128:(k + 1) * 128],
                    rhs=nmucol[:],
                    start=False, stop=True,
                )

            # sqrt batch (second ACT table set), then DVE recip -> ry, yhat
            yhat = persist.tile([128, N], BF16)
            sqny = persist.tile([C, N], F32)
            ry_bc = persist.tile([C, N], F32)
            for h in range(2):
                nc.scalar.activation(sqny[:, h * HALF:(h + 1) * HALF],
                                     pss[h][:C, :], AF.Sqrt)
            sqnx = small.tile([128, CH], F32, tag="sqnx")
            nc.scalar.activation(
                sqnx[:], nx2[:].rearrange("p (k two) -> p k two", two=2)[:, :, 0],
                AF.Sqrt,
            )
            # cpm = -0.2002/rx5 = -1.001*sqrt(nx2); aa = -0.2/(pm + cpm)
            cpm = persist.tile([128, CH], F32)
            nc.vector.tensor_scalar_mul(cpm[:], sqnx[:], -(1.0 + EPS_MIN))
            QW = HALF // 2
            for h in range(2):
                for q in range(2):
                    c0 = h * HALF + q * QW
                    nc.vector.reciprocal_approx_fast(
                        ry_bc[:, c0:c0 + QW], sqny[:, c0:c0 + QW])
                    nc.vector.scalar_tensor_tensor(
                        yhat[:C, c0:c0 + QW],
                        in0=y_sb[:, c0:c0 + QW],
                        scalar=mu[:],
                        in1=ry_bc[:, c0:c0 + QW],
                        op0=OP.subtract,
                        op1=OP.mult,
                    )
                    nc.sync.dma_start(out=yhat[C:, c0:c0 + QW],
                                      in_=yhat[:C, c0:c0 + QW])

            # ---------------- main loop: chunk pairs via PE row tiling --------
            ssums = persist.tile([128, 2 * CH], F32)
            for kp in range(CH // 2):
                kA, kB = 2 * kp, 2 * kp + 1
                lhsA = xcen[0:C, kA * 128:(kA + 1) * 128]
                lhsB = xcen[C:128, kB * 128:(kB + 1) * 128]
                pmP = small.tile([128, 4], F32, tag="pmP", name="pmP")
                rbA = rbpool.tile([128, N], F32, tag="rbA", name="rbA")
                rbB = rbpool.tile([128, N], F32, tag="rbB", name="rbB")
                rbts = [rbA, rbB]
                psAs, psBs = [], []
                for h in range(2):
                    psA = mmpool.tile([128, HALF], F32, tag="mmA", name="psA")
                    psB = mmpool.tile([128, HALF], F32, tag="mmB", name="psB")
                    for j in range(4):
                        c0 = h * HALF + j * 512
                        nc.tensor.matmul(
                            psA[:, j * 512:(j + 1) * 512],
                            lhsT=lhsA,
                            rhs=yhat[0:C, c0:c0 + 512],
                            start=True, stop=True,
                            tile_position=(0, 0),
                        )
                        nc.tensor.matmul(
                            psB[:, j * 512:(j + 1) * 512],
                            lhsT=lhsB,
                            rhs=yhat[C:128, c0:c0 + 512],
                            start=True, stop=True,
                            tile_position=(64, 0),
                        )
                    psAs.append(psA)
                    psBs.append(psB)

                for i, (k, pss, rbt) in enumerate(
                        ((kA, psAs, rbA), (kB, psBs, rbB))):
                    pm = pmP[:, 2 * i:2 * i + 2]
                    for h in range(2):
                        init = -3.0e38 if h == 0 else pm[:, 0:1]
                        # rb = copy(ps); pm = max(row-max(ps), init)
                        nc.vector._custom_dve(
                            TENSOR_MASK_REDUCE,
                            out=rbt[:, h * HALF:(h + 1) * HALF],
                            in0=pss[h][:],
                            in1=c3big[:],
                            s0=0.0,
                            s1=init,
                            imm2=1.0,
                            accum_out=pm[:, h:h + 1],
                        )
                    # per-row constants: aa = -0.2/(pm + c), c = -0.2002/rx5,
                    # bb = -aa*pm  (gpsimd + fast reciprocal on DVE)
                    pmx = pm[:, 1:2]
                    tden = small.tile([128, 1], F32, tag=f"tden{i}", name="tden")
                    nc.gpsimd.tensor_tensor(tden[:], pmx, cpm[:, k:k + 1], OP.add)
                    rec = small.tile([128, 1], F32, tag=f"rec{i}", name="rec")
                    nc.vector.reciprocal_approx_fast(rec[:], tden[:])
                    aa = small.tile([128, 1], F32, tag=f"aa{i}", name="aa")
                    nc.gpsimd.tensor_scalar_mul(aa[:], rec[:], -0.2)
                    bbt = small.tile([128, 1], F32, tag=f"bbt{i}", name="bbt")
                    nc.gpsimd.tensor_tensor(bbt[:], aa[:], pmx, OP.mult)
                    bb = small.tile([128, 1], F32, tag=f"bb{i}", name="bb")
                    nc.gpsimd.tensor_scalar_mul(bb[:], bbt[:], -1.0)
                    for h in range(2):
                        es = espool.tile([128, HALF], BF16, tag="es", name="es")
                        nc.scalar.activation(
                            es[:], rbt[:, h * HALF:(h + 1) * HALF], AF.Exp,
                            bias=bb[:], scale=aa[:],
                            accum_out=ssums[:, 2 * k + h:2 * k + h + 1],
                        )

            # ---------------- finalize ----------------
            nc.sync.dma_start(out=out_d[:], in_=ssums[:])

    nc.compile()
    return nc


def _get_nc():
    if "nc" not in _NC_CACHE:
        _NC_CACHE["nc"] = build_nc()
    return _NC_CACHE["nc"]


def make_in_maps(X_features, Y_features):
    X = np.ascontiguousarray(np.asarray(X_features, np.float32).reshape(B, C, N))
    Y = np.ascontiguousarray(np.asarray(Y_features, np.float32).reshape(B, C, N))
    in_maps = []
    for c in range(8):
        b, h = divmod(c, 2)
        in_maps.append({
            "Xh": np.ascontiguousarray(X[b, :, h * NX:(h + 1) * NX]),
            "Yb": Y[b],
        })
    return in_maps


def combine(results):
    """results: list of 8 dicts with 'out' [128, CH] = S' per row."""
    out = np.empty(B, np.float32)
    for b in range(B):
        tot = 0.0
        for h in range(2):
            s2 = results[2 * b + h]["out"].astype(np.float64)
            s = s2[:, 0::2] + s2[:, 1::2]
            tot += (1.0 / s).sum()
        out[b] = -np.log(tot / N)
    return out


def kernel(X_features, Y_features):
    nc = _get_nc()
    in_maps = make_in_maps(X_features, Y_features)
    res = run_bass_kernel_spmd(nc, in_maps, core_ids=list(range(8)))
    return combine(res.results)


if __name__ == "__main__":
    rng = np.random.default_rng(0)
    X = rng.standard_normal((B, C, 64, 64)).astype(np.float32)
    Y = rng.standard_normal((B, C, 64, 64)).astype(np.float32)
    print(kernel(X_features=X, Y_features=Y))
